# revision 20
# baseline (speedup 1.0000x reference)
"""Trainium2 Bass kernel for MinimalHGRNCore (BitLinear projections + HGRN scan).

Contract: kernel(**inputs) takes FULL unsharded numpy inputs and returns the
FULL (B, L, H) float32 output.

Sharding: 8 cores = (batch b in 0..3) x (E-half eh in 0..1).
Each core processes all L tokens of one batch and half of the E features for
the i/f/g projections + recurrence; the final Wo projection is split by
output-H half, contracting over full E via a pair-wise AllGather of the
quantized y activations.

Exactness: act_quant produces integers in [-127,127] and weight_quant values
in {-1,0,+1} * scales.  Both are exactly representable in fp16, so the PE
matmuls run in fp16 with fp32 PSUM accumulation == exact integer arithmetic
(|sum| <= 2048*127 < 2^24).  Rounding uses the fp16 magic-number trick
(x + 1536 downcast to fp16 rounds to nearest-even integer for |x| <= 510),
matching jnp.round's half-to-even semantics.
"""

from contextlib import ExitStack
from dataclasses import dataclass

import numpy as np

import concourse.bass as bass
import concourse.mybir as mybir
import concourse.tile as tile
from concourse import bacc
from concourse.masks import make_identity

F32 = mybir.dt.float32
F16 = mybir.dt.float16
AF = mybir.ActivationFunctionType
ALU = mybir.AluOpType
AX = mybir.AxisListType

M32 = 12582912.0  # 1.5 * 2**23: fp32 add rounds to nearest-even integer exactly


@dataclass
class Cfg:
    T: int = 2048      # tokens per core (= L of its batch)
    H: int = 2048      # input hidden dim (contraction for i/f/g)
    EL: int = 1024     # local E features per core (= E/2)
    n_cores: int = 8
    silu_lut: bool = True  # False: decompose silu=z*sigmoid(z) (CoreSim lacks Silu)

    @property
    def E(self):
        return 2 * self.EL

    @property
    def HL(self):
        return self.H // 2

    @property
    def MT(self):
        return self.T // 128

    @property
    def KH(self):
        return self.H // 128

    @property
    def JE(self):
        return self.EL // 128

    @property
    def KE(self):
        return self.E // 128

    @property
    def NT(self):
        return min(512, self.T)

    @property
    def NN(self):
        return self.T // self.NT

    @property
    def NH(self):
        return min(512, self.HL)

    @property
    def NHN(self):
        return self.HL // self.NH

    @property
    def pairs(self):
        return [[2 * i, 2 * i + 1] for i in range(self.n_cores // 2)]


def build_hgrn(tc: tile.TileContext, outs: dict, ins: dict, cfg: Cfg):
    """Emit the SPMD program (identical on every core) into TileContext tc."""
    nc = tc.nc
    c = cfg
    x, wiT, wfT, wgT, woT = ins["x"], ins["wiT"], ins["wfT"], ins["wgT"], ins["woT"]
    rms_w_h, norm_o_h = ins["rms_w_h"], ins["norm_o_h"]
    out = outs["out"]

    ctx = ExitStack()
    with ctx:
        # whole-kernel pools: constants + small stat tiles
        const = ctx.enter_context(tc.tile_pool(name="const", bufs=1))
        small = ctx.enter_context(tc.tile_pool(name="small", bufs=2))
        dram = ctx.enter_context(tc.tile_pool(name="dram", bufs=1, space="DRAM"))

        ones_row = const.tile([1, 128], F32, tag="ones_row")
        nc.vector.memset(ones_row[:], 1.0)
        ones_col = const.tile([128, 1], F32, tag="ones_col")
        nc.vector.memset(ones_col[:], 1.0)
        ident = const.tile([128, 128], F32, tag="ident")
        make_identity(nc, ident[:])

        norm_o_row = const.tile([1, c.EL], F32, tag="norm_o_row")
        nc.sync.dma_start(norm_o_row[0:1, :],
                          norm_o_h[:].rearrange("(a t) -> a t", a=1))
        rms_cols = const.tile([128, c.JE], F32, tag="rms_cols")
        norm_o_cols = const.tile([128, c.JE], F32, tag="norm_o_cols")
        nc.sync.dma_start(rms_cols[:],
                          rms_w_h[:].rearrange("(j p) -> p j", p=128))
        nc.sync.dma_start(norm_o_cols[:],
                          norm_o_h[:].rearrange("(j p) -> p j", p=128))
        no2_cols = const.tile([128, c.JE], F32, tag="no2_cols")
        nc.vector.tensor_tensor(no2_cols[:], norm_o_cols[:], norm_o_cols[:],
                                ALU.mult)

        # ------------------------------------------------------------------
        # P1: |W| sums -> pair AllGather -> per-tensor scales
        # ------------------------------------------------------------------
        cc1a_in = dram.tile([1, 3], F32, tag="cc1a_in")
        cc1a_out = dram.tile([2, 3], F32, tag="cc1a_out")
        cc1b_in = dram.tile([1, 1], F32, tag="cc1b_in")
        cc1b_out = dram.tile([2, 1], F32, tag="cc1b_out")

        wsums = const.tile([1, 4], F32, tag="wsums")

        def w_abs_sum(wT, n_row_chunks, row_elems, idx, pool):
            # balanced 128-block tree reductions to track jax's pairwise
            # fp32 summation closely (the scale feeds round(); ~1e-5 rel
            # error here flips ternary weights)
            cols = small.tile([128, n_row_chunks], F32, tag="wabs_cols")
            nblk = row_elems // 128
            for r in range(n_row_chunks):
                wt = pool.tile([128, row_elems], F32, tag="wabs_t")
                nc.sync.dma_start(wt[:], wT[r * 128 : (r + 1) * 128, :])
                blk = small.tile([128, nblk], F32, tag="wabs_blk")
                nc.vector.tensor_reduce(
                    blk[:], wt[:].rearrange("p (a b) -> p a b", b=128),
                    AX.X, ALU.add, apply_absolute_value=True)
                nc.vector.tensor_reduce(cols[:, r : r + 1], blk[:], AX.X,
                                        ALU.add)
            tot = small.tile([128, 1], F32, tag="wabs_tot")
            nc.vector.tensor_reduce(tot[:], cols[:], AX.X, ALU.add)
            row = small.tile([1, 128], F32, tag="wabs_row")
            with tc.tile_pool(name="wabs_ps", bufs=1, space="PSUM") as pp:
                ps = pp.tile([1, 128], F32, tag="wabs_ps")
                nc.tensor.transpose(ps[:], tot[:], ident[:])
                nc.scalar.copy(row[:], ps[:])
            r16 = small.tile([1, 16], F32, tag="wabs_r16")
            nc.vector.tensor_reduce(r16[:],
                                    row[:].rearrange("p (a b) -> p a b", b=8),
                                    AX.X, ALU.add)
            nc.vector.tensor_reduce(wsums[0:1, idx : idx + 1], r16[:], AX.X,
                                    ALU.add)

        with tc.tile_pool(name="wabs", bufs=2) as wpool:
            for i_w, wT in enumerate([wiT, wfT, wgT]):
                w_abs_sum(wT, c.KH, c.EL, i_w, wpool)
            nc.sync.dma_start(cc1a_in[:], wsums[0:1, 0:3])
            nc.gpsimd.collective_compute(
                "AllGather", ALU.bypass, replica_groups=c.pairs,
                ins=[cc1a_in.opt()], outs=[cc1a_out.opt()])
            w_abs_sum(woT, c.KE, c.HL, 3, wpool)
            nc.sync.dma_start(cc1b_in[:], wsums[0:1, 3:4])
            nc.gpsimd.collective_compute(
                "AllGather", ALU.bypass, replica_groups=c.pairs,
                ins=[cc1b_in.opt()], outs=[cc1b_out.opt()])

        wsum_a = const.tile([1, 4], F32, tag="wsum_a")
        wsum_b = const.tile([1, 4], F32, tag="wsum_b")
        nc.sync.dma_start(wsum_a[0:1, 0:3], cc1a_out[0:1, :])
        nc.sync.dma_start(wsum_b[0:1, 0:3], cc1a_out[1:2, :])
        nc.sync.dma_start(wsum_a[0:1, 3:4], cc1b_out[0:1, :])
        nc.sync.dma_start(wsum_b[0:1, 3:4], cc1b_out[1:2, :])
        m_w = const.tile([1, 4], F32, tag="m_w")
        nc.vector.tensor_tensor(m_w[:], wsum_a[:], wsum_b[:], ALU.add)
        n_w_elems = float(c.H) * float(c.E)
        nc.vector.tensor_scalar(m_w[:], m_w[:], 1.0 / n_w_elems, 1e-5,
                                ALU.mult, ALU.max)
        s_w = const.tile([1, 4], F32, tag="s_w")
        nc.vector.reciprocal(s_w[:], m_w[:])

        def bcast_col(src_ap, tag):
            t = const.tile([128, 1], F32, tag=tag)
            with tc.tile_pool(name="bc_ps", bufs=1, space="PSUM") as pp:
                ps = pp.tile([128, 1], F32, tag="bc_ps")
                nc.tensor.matmul(ps[:], ones_row[:], src_ap, start=True, stop=True)
                nc.scalar.copy(t[:], ps[:])
            return t

        s_wi_c = bcast_col(s_w[0:1, 0:1], "s_wi")
        s_wf_c = bcast_col(s_w[0:1, 1:2], "s_wf")
        s_wg_c = bcast_col(s_w[0:1, 2:3], "s_wg")
        s_wo_c = bcast_col(s_w[0:1, 3:4], "s_wo")
        m_wi_c = bcast_col(m_w[0:1, 0:1], "m_wi")
        m_wf_c = bcast_col(m_w[0:1, 1:2], "m_wf")
        m_wg_c = bcast_col(m_w[0:1, 2:3], "m_wg")
        m_wo_c = bcast_col(m_w[0:1, 3:4], "m_wo")
        nm_wf_c = const.tile([128, 1], F32, tag="nm_wf")
        nc.vector.tensor_scalar(nm_wf_c[:], m_wf_c[:], -1.0, None, ALU.mult)

        d_all = const.tile([128, c.MT], F32, tag="d_all")  # 1/scale_tok cols
        u_dram = dram.tile([c.EL, c.T], F32, tag="u_dram")
        cc2_in = dram.tile([3, c.T], F32, tag="cc2_in")
        cc2_out = dram.tile([2, 3, c.T], F32, tag="cc2_out")

        ssq_s_cols = const.tile([128, c.MT], F32, tag="ssq_s_cols")
        ssq_u_cols = const.tile([128, c.MT], F32, tag="ssq_u_cols")
        vmax_cols = const.tile([128, c.MT], F32, tag="vmax_cols")

        # ===== stats accumulators + dq broadcast: live P2..P4a =====
        with tc.tile_pool(name="stats", bufs=1) as stats:
            dq_b = stats.tile([128, c.T], F32, tag="dq_b")
            sq_acc_s = stats.tile([128, c.T], F32, tag="sq_acc_s")
            sq_acc_u = stats.tile([128, c.T], F32, tag="sq_acc_u")
            vmax = stats.tile([128, c.T], F32, tag="vmax")

            # ===== xqT: quantized transposed activations, live P2..P3 =====
            with tc.tile_pool(name="xqTp", bufs=1) as xqTp:
                xqT_t = xqTp.tile([128, c.KH, c.T], F16, tag="xqT")
                xq_dram = dram.tile([c.T, c.H], F16, tag="xq_dram")

                # ----------------------------------------------------------
                # P2: x stats + act_quant + DMA-transpose
                # ----------------------------------------------------------
                with tc.tile_pool(name="xphase", bufs=2) as xp, \
                     tc.tile_pool(name="xsq", bufs=1) as xsqp, \
                     tc.tile_pool(name="xq16", bufs=2) as xqp:
                    for m in range(c.MT):
                        xt = xp.tile([128, c.H], F32, tag="x_t")
                        nc.sync.dma_start(xt[:], x[m * 128 : (m + 1) * 128, :])
                        sq = xsqp.tile([128, c.H], F32, tag="x_sq")
                        nc.scalar.activation(sq[:], xt[:], AF.Square)
                        sblk = small.tile([128, c.KH], F32, tag="x_sblk")
                        nc.vector.tensor_reduce(
                            sblk[:], sq[:].rearrange("p (a b) -> p a b", b=128),
                            AX.X, ALU.add)
                        ssq = small.tile([128, 1], F32, tag="x_ssq")
                        nc.vector.tensor_reduce(ssq[:], sblk[:], AX.X, ALU.add)
                        amax = small.tile([128, 1], F32, tag="x_amax")
                        nc.vector.tensor_reduce(amax[:], xt[:], AX.X, ALU.max,
                                                apply_absolute_value=True)
                        v = small.tile([128, 1], F32, tag="x_v")
                        nc.vector.tensor_scalar(v[:], ssq[:], 1.0 / c.H, 1e-8,
                                                ALU.mult, ALU.add)
                        rv = small.tile([128, 1], F32, tag="x_rv")
                        nc.vector.reciprocal(rv[:], v[:])
                        r0 = small.tile([128, 1], F32, tag="x_r0")
                        nc.scalar.sqrt(r0[:], rv[:])
                        # Newton: r = r0*(1.5 - 0.5*v*r0^2) -> ~1ulp rsqrt(v)
                        nt = small.tile([128, 1], F32, tag="x_nt")
                        nc.vector.tensor_tensor(nt[:], r0[:], r0[:], ALU.mult)
                        nc.vector.tensor_tensor(nt[:], nt[:], v[:], ALU.mult)
                        nc.vector.tensor_scalar(nt[:], nt[:], -0.5, 1.5,
                                                ALU.mult, ALU.add)
                        rstd = small.tile([128, 1], F32, tag="x_rstd")
                        nc.vector.tensor_tensor(rstd[:], r0[:], nt[:], ALU.mult)
                        amx = small.tile([128, 1], F32, tag="x_amx")
                        nc.vector.tensor_tensor(amx[:], amax[:], rstd[:], ALU.mult)
                        nc.vector.tensor_scalar(amx[:], amx[:], 1e-5, None,
                                                ALU.max)
                        ra = small.tile([128, 1], F32, tag="x_ra")
                        nc.vector.reciprocal(ra[:], amx[:])
                        sc = small.tile([128, 1], F32, tag="x_sc")
                        nc.vector.tensor_scalar(sc[:], ra[:], 127.0, None,
                                                ALU.mult)
                        cc = small.tile([128, 1], F32, tag="x_cc")
                        nc.vector.tensor_tensor(cc[:], sc[:], rstd[:], ALU.mult)
                        nc.vector.reciprocal(d_all[:, m : m + 1], sc[:])

                        xqf = xp.tile([128, c.H], F32, tag="xqf")
                        nc.scalar.activation(xqf[:], xt[:], AF.Copy, bias=M32,
                                             scale=cc[:])
                        xq = xqp.tile([128, c.H], F16, tag="xq16")
                        nc.vector.tensor_scalar(xq[:], xqf[:], M32, None,
                                                ALU.subtract)
                        nc.sync.dma_start(xq_dram[m * 128 : (m + 1) * 128, :],
                                          xq[:])
                        if "d1_xq" in outs:
                            nc.sync.dma_start(
                                outs["d1_xq"][m * 128 : (m + 1) * 128, :], xq[:])

                    # one big transposing DMA: xqT[p, k, t] = xq[t, k*128+p]
                    nc.sync.dma_start_transpose(xqT_t[:], xq_dram[:, :])

                    # d columns -> d row -> dq broadcast tile [128, T]
                    dscr = dram.tile([c.T], F32, tag="dscr")
                    nc.sync.dma_start(
                        dscr[:].rearrange("(m p) -> p m", p=128), d_all[:])
                    d_row = xp.tile([1, c.T], F32, tag="d_row")
                    nc.sync.dma_start(d_row[0:1, :],
                                      dscr[:].rearrange("(a t) -> a t", a=1))
                    with tc.tile_pool(name="dq_ps", bufs=2, space="PSUM") as pp:
                        for q in range(c.T // c.NT):
                            ps = pp.tile([128, c.NT], F32, tag="dq_ps")
                            nc.tensor.matmul(
                                ps[:], ones_row[:],
                                d_row[0:1, q * c.NT : (q + 1) * c.NT],
                                start=True, stop=True)
                            nc.scalar.copy(dq_b[:, q * c.NT : (q + 1) * c.NT],
                                           ps[:])

                # ----------------------------------------------------------
                # P3: per local-e chunk: W strips, i/f/g matmuls, gates,
                #     scan, u, stat accumulation.  u spilled to DRAM.
                # ----------------------------------------------------------
                with tc.tile_pool(name="p3", bufs=2) as p3, \
                     tc.tile_pool(name="p3w", bufs=2) as p3w, \
                     tc.tile_pool(name="p3q", bufs=4) as p3q, \
                     tc.tile_pool(name="p3s", bufs=2) as p3s, \
                     tc.tile_pool(name="p3a", bufs=3) as p3a, \
                     tc.tile_pool(name="mm_ps", bufs=4, space="PSUM") as mm_ps:

                    KHH = max(1, c.KH // 2)  # quantize strips in k-halves

                    def quant_w_strip(wT, s_col, j, nm):
                        q = p3q.tile([128, c.KH, 128], F16, tag="wq_strip",
                                     name=nm)
                        for kh in range(0, c.KH, KHH):
                            strip = p3w.tile([128, KHH, 128], F32, tag="w_strip",
                                             name="w_strip")
                            nc.sync.dma_start(
                                strip[:],
                                wT[kh * 128 : (kh + KHH) * 128,
                                   j * 128 : (j + 1) * 128]
                                .rearrange("(k p) e -> p k e", p=128))
                            sv = strip[:].rearrange("p k e -> p (k e)")
                            qv = q[:, kh : kh + KHH, :].rearrange(
                                "p k e -> p (k e)")
                            # fp32 magic: v + M32 rounds v to int (half-even)
                            nc.vector.tensor_scalar(sv, sv, s_col[:], M32,
                                                    ALU.mult, ALU.add)
                            nc.vector.tensor_scalar(sv, sv, M32, 1.0,
                                                    ALU.subtract, ALU.min)
                            nc.vector.tensor_scalar(qv, sv, -1.0, None, ALU.max)
                        return q

                    def proj_psum(wq, n):
                        ps = mm_ps.tile([128, c.NT], F32, tag="proj_ps",
                                        name="proj_ps")
                        for k in range(c.KH):
                            nc.tensor.matmul(
                                ps[:], wq[:, k, :],
                                xqT_t[:, k, n * c.NT : (n + 1) * c.NT],
                                start=(k == 0), stop=(k == c.KH - 1))
                        return ps

                    for j in range(c.JE):
                        wq_f = quant_w_strip(wfT, s_wf_c, j, "wq_f")
                        wq_i = quant_w_strip(wiT, s_wi_c, j, "wq_i")
                        wq_g = quant_w_strip(wgT, s_wg_c, j, "wq_g")

                        # big tiles: tag A holds {f, u}, tag B holds {ii, s}
                        f_j = p3.tile([128, c.T], F32, tag="bigA", name="f_j")
                        ii_j = p3.tile([128, c.T], F32, tag="bigB", name="ii_j")
                        for n in range(c.NN):
                            sl = bass.ts(n, c.NT)
                            ps_f = proj_psum(wq_f, n)
                            t_f = p3s.tile([128, c.NT], F32, tag="t_raw",
                                           name="t_f")
                            nc.vector.tensor_tensor(t_f[:], ps_f[:], dq_b[:, sl],
                                                    ALU.mult)
                            nc.scalar.activation(f_j[:, sl], t_f[:], AF.Sigmoid,
                                                 scale=m_wf_c[:])
                            fm = p3a.tile([128, c.NT], F32, tag="act_o",
                                          name="fm")
                            nc.scalar.activation(fm[:], t_f[:], AF.Sigmoid,
                                                 scale=nm_wf_c[:])
                            ps_i = proj_psum(wq_i, n)
                            t_i = p3s.tile([128, c.NT], F32, tag="t_raw",
                                           name="t_i")
                            nc.vector.tensor_tensor(t_i[:], ps_i[:], dq_b[:, sl],
                                                    ALU.mult)
                            si = p3a.tile([128, c.NT], F32, tag="act_o",
                                          name="si")
                            if c.silu_lut:
                                nc.scalar.activation(si[:], t_i[:], AF.Silu,
                                                     scale=m_wi_c[:])
                            else:
                                zi = p3a.tile([128, c.NT], F32, tag="z_t",
                                              name="zi")
                                nc.vector.tensor_scalar(zi[:], t_i[:], m_wi_c[:],
                                                        None, ALU.mult)
                                nc.scalar.activation(si[:], zi[:], AF.Sigmoid)
                                nc.vector.tensor_tensor(si[:], si[:], zi[:],
                                                        ALU.mult)
                            nc.vector.tensor_tensor(ii_j[:, sl], si[:], fm[:],
                                                    ALU.mult)

                        s_j = p3.tile([128, c.T], F32, tag="bigB", name="s_j")
                        nc.vector.tensor_tensor_scan(s_j[:], f_j[:], ii_j[:],
                                                     0.0, ALU.mult, ALU.add)
                        if "d2_f" in outs:
                            nc.sync.dma_start(
                                outs["d2_f"][j * 128 : (j + 1) * 128, :], f_j[:])
                        if "d3_s" in outs:
                            nc.sync.dma_start(
                                outs["d3_s"][j * 128 : (j + 1) * 128, :], s_j[:])

                        u_j = p3.tile([128, c.T], F32, tag="bigA", name="u_j")
                        for n in range(c.NN):
                            sl = bass.ts(n, c.NT)
                            ps_g = proj_psum(wq_g, n)
                            t_g = p3s.tile([128, c.NT], F32, tag="t_raw",
                                           name="t_g")
                            nc.vector.tensor_tensor(t_g[:], ps_g[:], dq_b[:, sl],
                                                    ALU.mult)
                            gg = p3a.tile([128, c.NT], F32, tag="act_o",
                                          name="gg")
                            if c.silu_lut:
                                nc.scalar.activation(gg[:], t_g[:], AF.Silu,
                                                     scale=m_wg_c[:])
                            else:
                                zg = p3a.tile([128, c.NT], F32, tag="z_t",
                                              name="zg")
                                nc.vector.tensor_scalar(zg[:], t_g[:], m_wg_c[:],
                                                        None, ALU.mult)
                                nc.scalar.activation(gg[:], zg[:], AF.Sigmoid)
                                nc.vector.tensor_tensor(gg[:], gg[:], zg[:],
                                                        ALU.mult)
                            # u = gg * s * rms_w
                            nc.vector.tensor_tensor(u_j[:, sl], gg[:],
                                                    s_j[:, sl], ALU.mult)
                            nc.vector.tensor_scalar(u_j[:, sl], u_j[:, sl],
                                                    rms_cols[:, j : j + 1],
                                                    None, ALU.mult)

                            # stats: ssq_s, ssq_u accumulate; vmax tree
                            sq1 = p3s.tile([128, c.NT], F32, tag="sq_scr",
                                           name="sq1")
                            nc.scalar.activation(sq1[:], s_j[:, sl], AF.Square)
                            if j == 0:
                                nc.vector.tensor_copy(sq_acc_s[:, sl], sq1[:])
                            else:
                                nc.vector.tensor_tensor(sq_acc_s[:, sl],
                                                        sq_acc_s[:, sl], sq1[:],
                                                        ALU.add)
                            sq2 = p3s.tile([128, c.NT], F32, tag="sq_scr",
                                           name="sq2")
                            nc.scalar.activation(sq2[:], u_j[:, sl], AF.Square)
                            if j == 0:
                                nc.vector.tensor_copy(sq_acc_u[:, sl], sq2[:])
                                nc.vector.tensor_scalar(vmax[:, sl], sq2[:],
                                                        no2_cols[:, 0:1], None,
                                                        ALU.mult)
                            else:
                                nc.vector.tensor_tensor(sq_acc_u[:, sl],
                                                        sq_acc_u[:, sl], sq2[:],
                                                        ALU.add)
                                va2 = p3s.tile([128, c.NT], F32, tag="sq_scr",
                                               name="va2")
                                nc.vector.tensor_scalar(va2[:], sq2[:],
                                                        no2_cols[:, j : j + 1],
                                                        None, ALU.mult)
                                nc.vector.tensor_tensor(vmax[:, sl],
                                                        vmax[:, sl], va2[:],
                                                        ALU.max)
                        nc.sync.dma_start(u_dram[j * 128 : (j + 1) * 128, :],
                                          u_j[:])
                        if "d4_u" in outs:
                            nc.sync.dma_start(
                                outs["d4_u"][j * 128 : (j + 1) * 128, :], u_j[:])

            # --------------------------------------------------------------
            # P4a: partition-reduce stats -> per-token columns (in `const`)
            # --------------------------------------------------------------
            with tc.tile_pool(name="tp_ps", bufs=2, space="PSUM") as tpp:
                for src, dst, op in ((sq_acc_s, ssq_s_cols, ALU.add),
                                     (sq_acc_u, ssq_u_cols, ALU.add),
                                     (vmax, vmax_cols, ALU.max)):
                    for m in range(c.MT):
                        tp = tpp.tile([128, 128], F32, tag="tp_ps", name="tp")
                        nc.tensor.transpose(
                            tp[:], src[:, m * 128 : (m + 1) * 128], ident[:])
                        nc.vector.tensor_reduce(dst[:, m : m + 1], tp[:], AX.X,
                                                op)
        # stats pool closed here

        amax_cols = const.tile([128, c.MT], F32, tag="amax_cols")
        a0 = const.tile([128, c.MT], F32, tag="amax_a0")
        nc.scalar.sqrt(a0[:], vmax_cols[:])
        # Newton sqrt: a = 0.5*(a0 + v/a0); sqrt(0)=0 guard via max on a0
        ar = const.tile([128, c.MT], F32, tag="amax_ar")
        nc.vector.tensor_scalar(ar[:], a0[:], 1e-30, None, ALU.max)
        nc.vector.reciprocal(ar[:], ar[:])
        nc.vector.tensor_tensor(ar[:], ar[:], vmax_cols[:], ALU.mult)
        nc.vector.tensor_tensor(ar[:], ar[:], a0[:], ALU.add)
        nc.vector.tensor_scalar(amax_cols[:], ar[:], 0.5, None, ALU.mult)

        for row, cols in ((0, ssq_s_cols), (1, ssq_u_cols), (2, amax_cols)):
            nc.sync.dma_start(cc2_in[row, :].rearrange("(m p) -> p m", p=128),
                              cols[:])
        nc.gpsimd.collective_compute(
            "AllGather", ALU.bypass, replica_groups=c.pairs,
            ins=[cc2_in.opt()], outs=[cc2_out.opt()])

        def load_stat_cols(row, op, tag):
            a = small.tile([128, c.MT], F32, tag=tag + "_a", name=tag + "_a")
            b = small.tile([128, c.MT], F32, tag=tag + "_b", name=tag + "_b")
            nc.sync.dma_start(a[:],
                              cc2_out[0, row, :].rearrange("(m p) -> p m", p=128))
            nc.sync.dma_start(b[:],
                              cc2_out[1, row, :].rearrange("(m p) -> p m", p=128))
            r = small.tile([128, c.MT], F32, tag=tag, name=tag)
            nc.vector.tensor_tensor(r[:], a[:], b[:], op)
            return r

        def refine_rsqrt_cols(v_ap, r0_ap, out_ap, tag):
            nt = small.tile([128, c.MT], F32, tag=tag)
            nc.vector.tensor_tensor(nt[:], r0_ap, r0_ap, ALU.mult)
            nc.vector.tensor_tensor(nt[:], nt[:], v_ap, ALU.mult)
            nc.vector.tensor_scalar(nt[:], nt[:], -0.5, 1.5, ALU.mult, ALU.add)
            nc.vector.tensor_tensor(out_ap, r0_ap, nt[:], ALU.mult)

        ssq_s = load_stat_cols(0, ALU.add, "ssq_s")
        ssq_u = load_stat_cols(1, ALU.add, "ssq_u")
        amax_y = load_stat_cols(2, ALU.max, "amax_y")

        ms = small.tile([128, c.MT], F32, tag="ms")
        nc.vector.tensor_scalar(ms[:], ssq_s[:], 1.0 / c.E, 1e-5, ALU.mult,
                                ALU.add)
        rms_i = small.tile([128, c.MT], F32, tag="rms_i")
        nc.vector.reciprocal(rms_i[:], ms[:])
        rstd_s0 = small.tile([128, c.MT], F32, tag="rstd_s0")
        nc.scalar.sqrt(rstd_s0[:], rms_i[:])
        rstd_s = small.tile([128, c.MT], F32, tag="rstd_s")
        refine_rsqrt_cols(ms[:], rstd_s0[:], rstd_s[:], "nt_s")

        m2 = small.tile([128, c.MT], F32, tag="m2")
        nc.vector.tensor_scalar(m2[:], ssq_u[:], 1.0 / c.E, None, ALU.mult)
        r2 = small.tile([128, c.MT], F32, tag="r2")
        nc.vector.tensor_tensor(r2[:], rstd_s[:], rstd_s[:], ALU.mult)
        nc.vector.tensor_tensor(m2[:], m2[:], r2[:], ALU.mult)
        nc.vector.tensor_scalar(m2[:], m2[:], 1e-8, None, ALU.add)
        m2i = small.tile([128, c.MT], F32, tag="m2i")
        nc.vector.reciprocal(m2i[:], m2[:])
        rsty0 = small.tile([128, c.MT], F32, tag="rsty0")
        nc.scalar.sqrt(rsty0[:], m2i[:])
        rsty = small.tile([128, c.MT], F32, tag="rsty")
        refine_rsqrt_cols(m2[:], rsty0[:], rsty[:], "nt_y")

        rr = small.tile([128, c.MT], F32, tag="rr")
        nc.vector.tensor_tensor(rr[:], rstd_s[:], rsty[:], ALU.mult)
        av = small.tile([128, c.MT], F32, tag="av")
        nc.vector.tensor_tensor(av[:], amax_y[:], rr[:], ALU.mult)
        nc.vector.tensor_scalar(av[:], av[:], 1e-5, None, ALU.max)
        avi = small.tile([128, c.MT], F32, tag="avi")
        nc.vector.reciprocal(avi[:], av[:])
        sc_y = small.tile([128, c.MT], F32, tag="sc_y")
        nc.vector.tensor_scalar(sc_y[:], avi[:], 127.0, None, ALU.mult)
        c_y = small.tile([128, c.MT], F32, tag="c_y")
        nc.vector.tensor_tensor(c_y[:], rr[:], sc_y[:], ALU.mult)
        d_y = const.tile([128, c.MT], F32, tag="d_y")
        nc.vector.reciprocal(d_y[:], sc_y[:])
        nc.vector.tensor_scalar(d_y[:], d_y[:], m_wo_c[:], None, ALU.mult)

        cscr = dram.tile([c.T], F32, tag="cscr")
        nc.sync.dma_start(cscr[:].rearrange("(m p) -> p m", p=128), c_y[:])

        # ------------------------------------------------------------------
        # P4b: quantize y (two CC halves), AllGather fp16
        # ------------------------------------------------------------------
        JH = max(1, c.JE // 2)
        cc3_in = [None, None]
        cc3_out = [None, None]
        n_half = [JH, c.JE - JH]
        for h in range(2):
            if n_half[h] == 0:
                continue
            cc3_in[h] = dram.tile([n_half[h] * 128, c.T], F16,
                                  tag=f"cc3_in{h}", name=f"cc3_in{h}")
            cc3_out[h] = dram.tile([2, n_half[h] * 128, c.T], F16,
                                   tag=f"cc3_out{h}", name=f"cc3_out{h}")

        with tc.tile_pool(name="yq", bufs=3) as yqp, \
             tc.tile_pool(name="nc_ps", bufs=2, space="PSUM") as ncp:
            c_row = const.tile([1, c.T], F32, tag="c_row")
            nc.sync.dma_start(c_row[0:1, :],
                              cscr[:].rearrange("(a t) -> a t", a=1))
            for h in range(2):
                if n_half[h] == 0:
                    continue
                for jj in range(n_half[h]):
                    j = h * JH + jj
                    yq_j = yqp.tile([128, c.T], F16, tag="yq_j", name="yq_j")
                    for n in range(c.NN):
                        sl = bass.ts(n, c.NT)
                        ps_nc = ncp.tile([128, c.NT], F32, tag="nc_ps",
                                         name="nc_ps")
                        nc.tensor.matmul(
                            ps_nc[:],
                            norm_o_row[0:1, j * 128 : (j + 1) * 128],
                            c_row[0:1, sl], start=True, stop=True)
                        u_rd = yqp.tile([128, c.NT], F32, tag="u_rd",
                                        name="u_rd")
                        nc.sync.dma_start(u_rd[:],
                                          u_dram[j * 128 : (j + 1) * 128, sl])
                        q0 = yqp.tile([128, c.NT], F32, tag="q0", name="q0")
                        nc.vector.tensor_tensor(q0[:], u_rd[:], ps_nc[:],
                                                ALU.mult)
                        nc.vector.tensor_scalar(q0[:], q0[:], M32, None,
                                                ALU.add)
                        nc.vector.tensor_scalar(yq_j[:, sl], q0[:], M32,
                                                None, ALU.subtract)
                    nc.sync.dma_start(cc3_in[h][jj * 128 : (jj + 1) * 128, :],
                                      yq_j[:])
                    if "d5_yq" in outs:
                        nc.sync.dma_start(
                            outs["d5_yq"][(h * JH + jj) * 128 :
                                          (h * JH + jj + 1) * 128, :], yq_j[:])
                nc.gpsimd.collective_compute(
                    "AllGather", ALU.bypass, replica_groups=c.pairs,
                    ins=[cc3_in[h].opt()], outs=[cc3_out[h].opt()])

        # ------------------------------------------------------------------
        # P5: quantize Wo, final matmul over full E, dequant, store
        # ------------------------------------------------------------------
        with tc.tile_pool(name="woqp", bufs=1) as woq_p, \
             tc.tile_pool(name="yq_allp", bufs=1) as yq_p, \
             tc.tile_pool(name="wo_ld", bufs=2) as wol, \
             tc.tile_pool(name="out_sb", bufs=3) as osb, \
             tc.tile_pool(name="out_ps", bufs=4, space="PSUM") as ops:
            woq = woq_p.tile([128, c.KE, c.HL], F16, tag="woq")
            for k in range(c.KE):
                wt = wol.tile([128, c.HL], F32, tag="wo_t", name="wo_t")
                nc.sync.dma_start(wt[:], woT[k * 128 : (k + 1) * 128, :])
                nc.vector.tensor_scalar(wt[:], wt[:], s_wo_c[:], M32,
                                        ALU.mult, ALU.add)
                nc.vector.tensor_scalar(wt[:], wt[:], M32, 1.0,
                                        ALU.subtract, ALU.min)
                nc.vector.tensor_scalar(woq[:, k, :], wt[:], -1.0, None,
                                        ALU.max)

            yq_all = yq_p.tile([128, c.KE, c.T], F16, tag="yq_all")
            korder = []
            for h in range(2):
                if n_half[h] == 0:
                    continue
                for s in range(2):
                    for jj in range(n_half[h]):
                        kg = s * c.JE + h * JH + jj
                        korder.append((kg, h, s, jj))
            for kg, h, s, jj in korder:
                nc.sync.dma_start(yq_all[:, kg, :],
                                  cc3_out[h][s, jj * 128 : (jj + 1) * 128, :])

            for m in range(c.MT):
                msl = bass.ts(m, 128)
                for n in range(c.NHN):
                    nsl = bass.ts(n, c.NH)
                    ps = ops.tile([128, c.NH], F32, tag="out_ps", name="out_ps")
                    for ki, (kg, h, s, jj) in enumerate(korder):
                        nc.tensor.matmul(ps[:], yq_all[:, kg, msl],
                                         woq[:, kg, nsl],
                                         start=(ki == 0),
                                         stop=(ki == len(korder) - 1))
                    ot = osb.tile([128, c.NH], F32, tag="out_t", name="out_t")
                    nc.scalar.activation(ot[:], ps[:], AF.Copy,
                                         scale=d_y[:, m : m + 1])
                    nc.sync.dma_start(out[msl, nsl], ot[:])


# ----------------------------------------------------------------------
# Host wrapper
# ----------------------------------------------------------------------
_CACHE = {}


def _build_full_program(cfg: Cfg):
    nc = bacc.Bacc(None, target_bir_lowering=False, debug=False,
                   num_devices=cfg.n_cores)
    ins_h = {
        "x": nc.dram_tensor("x", [cfg.T, cfg.H], F32, kind="ExternalInput"),
        "wiT": nc.dram_tensor("wiT", [cfg.H, cfg.EL], F32, kind="ExternalInput"),
        "wfT": nc.dram_tensor("wfT", [cfg.H, cfg.EL], F32, kind="ExternalInput"),
        "wgT": nc.dram_tensor("wgT", [cfg.H, cfg.EL], F32, kind="ExternalInput"),
        "woT": nc.dram_tensor("woT", [cfg.E, cfg.HL], F32, kind="ExternalInput"),
        "rms_w_h": nc.dram_tensor("rms_w_h", [cfg.EL], F32, kind="ExternalInput"),
        "norm_o_h": nc.dram_tensor("norm_o_h", [cfg.EL], F32,
                                   kind="ExternalInput"),
    }
    out_h = nc.dram_tensor("out", [cfg.T, cfg.HL], F32, kind="ExternalOutput")
    outs = {"out": out_h[:, :]}
    import os
    if os.environ.get("HGRN_DEBUG"):
        for nm, shape, dt in (("d1_xq", [cfg.T, cfg.H], F16),
                              ("d2_f", [cfg.EL, cfg.T], F32),
                              ("d3_s", [cfg.EL, cfg.T], F32),
                              ("d4_u", [cfg.EL, cfg.T], F32),
                              ("d5_yq", [cfg.EL, cfg.T], F16)):
            h = nc.dram_tensor(nm, shape, dt, kind="ExternalOutput")
            outs[nm] = h[:, :]
    with tile.TileContext(nc) as tc:
        build_hgrn(tc, outs,
                   {k: v[tuple(slice(None) for _ in v.shape)]
                    for k, v in ins_h.items()}, cfg)
    nc.compile()
    return nc


def make_in_maps(x, Wi, Wf, Wg, Wo, rms_w, norm_o, cfg: Cfg):
    in_maps = []
    for core in range(cfg.n_cores):
        b, eh = core // 2, core % 2
        esl = slice(eh * cfg.EL, (eh + 1) * cfg.EL)
        hsl = slice(eh * cfg.HL, (eh + 1) * cfg.HL)
        in_maps.append({
            "x": np.ascontiguousarray(x[b]),
            "wiT": np.ascontiguousarray(Wi[esl, :].T),
            "wfT": np.ascontiguousarray(Wf[esl, :].T),
            "wgT": np.ascontiguousarray(Wg[esl, :].T),
            "woT": np.ascontiguousarray(Wo[hsl, :].T),
            "rms_w_h": np.ascontiguousarray(rms_w[esl]),
            "norm_o_h": np.ascontiguousarray(norm_o[esl]),
        })
    return in_maps


def kernel(x, Wi, Wf, Wg, Wo, norm_i, norm_f, norm_g, norm_o, rms_w,
           _trace=False):
    x = np.asarray(x, np.float32)
    for nv in (norm_i, norm_f, norm_g):
        if not np.allclose(np.asarray(nv), 1.0):
            raise NotImplementedError(
                "kernel assumes norm_i == norm_f == norm_g == 1 "
                "(as produced by setup_inputs)")
    B, L, H = x.shape
    cfg = Cfg(T=L, H=H, EL=np.asarray(Wi).shape[0] // 2, n_cores=8)
    assert B * 2 == cfg.n_cores

    from concourse import bass_utils

    key = (cfg.T, cfg.H, cfg.EL)
    if key not in _CACHE:
        _CACHE[key] = _build_full_program(cfg)
    nc = _CACHE[key]

    in_maps = make_in_maps(np.asarray(x, np.float32),
                           np.asarray(Wi, np.float32),
                           np.asarray(Wf, np.float32),
                           np.asarray(Wg, np.float32),
                           np.asarray(Wo, np.float32),
                           np.asarray(rms_w, np.float32),
                           np.asarray(norm_o, np.float32), cfg)
    res = bass_utils.run_bass_kernel_spmd(
        nc, in_maps, core_ids=list(range(cfg.n_cores)), trace=_trace)

    out = np.empty((B, L, H), np.float32)
    for core in range(cfg.n_cores):
        b, eh = core // 2, core % 2
        out[b, :, eh * cfg.HL : (eh + 1) * cfg.HL] = res.results[core]["out"]
    kernel.last_raw = res.results
    if _trace:
        kernel.last_exec_time_ns = res.exec_time_ns
        kernel.last_results = res
    return out
